# revision 11
# baseline (speedup 1.0000x reference)
"""HSE (hard squeeze-excite) Trainium2 Bass kernel.

Full inputs: x [32,56,56,256] f32, w1 [256,64], w2 [64,256].
out = x * hsigmoid(relu6(gap(x) @ w1) @ w2), gap = mean over H,W.

Sharding: pure data-parallel over batch, 4 samples per core on 8 cores.
Per-core layout: x shard [4*56*56, 256] tokens viewed as [128, 98, 256]
(partition p holds tokens p*98 .. p*98+97; 3136 = 32*98 so partitions
0-31 -> sample 0, 32-63 -> sample 1, etc. -- each partition line holds
tokens of exactly one sample).
"""

import numpy as np

B, H, W, C = 32, 56, 56, 256
CR = 64
NCORES = 8
BPC = B // NCORES            # 4 samples per core
TOK = H * W                  # 3136 tokens per sample
P = 128                      # SBUF partitions
TPP = BPC * TOK // P         # 98 tokens per partition
SPP = P // BPC               # 32 partitions per sample
CHUNK = 16                   # tokens per pipeline chunk (pow2 for tree adds)
NFULL = TPP // CHUNK         # 6 full chunks
REM = TPP - NFULL * CHUNK    # 2 leftover tokens

_CACHE = {}


def _build():
    import concourse.bacc as bacc
    import concourse.tile as tile
    import concourse.mybir as mybir

    f32 = mybir.dt.float32
    op = mybir.AluOpType

    nc = bacc.Bacc("TRN2", target_bir_lowering=False, debug=False)

    x_d = nc.dram_tensor("x", [P, TPP, C], f32, kind="ExternalInput").ap()
    w1_d = nc.dram_tensor("w1", [C, CR], f32, kind="ExternalInput").ap()
    w2_d = nc.dram_tensor("w2", [CR, C], f32, kind="ExternalInput").ap()
    mask_d = nc.dram_tensor("mask", [P, BPC], f32, kind="ExternalInput").ap()
    rt_d = nc.dram_tensor("rt", [BPC, P], f32, kind="ExternalInput").ap()
    o_d = nc.dram_tensor("out", [P, TPP, C], f32, kind="ExternalOutput").ap()

    with tile.TileContext(nc) as tc:
        with tc.tile_pool(name="big", bufs=1) as big, \
             tc.tile_pool(name="small", bufs=1) as small, \
             tc.tile_pool(name="psum", bufs=1, space="PSUM") as psum:

            X = big.tile([P, TPP, C], f32)          # whole shard, ~100KB/part
            acc = small.tile([P, C], f32)           # per-partition token sums
            w1s = small.tile([P, 2, CR], f32)       # w1 rows split in halves
            w2s = small.tile([CR, C], f32)
            mask = small.tile([P, BPC], f32)        # partition->sample (1/TOK)
            rt = small.tile([BPC, P], f32)          # sample->partition replicate
            sT_sb = small.tile([P, 2 * BPC], f32)
            zT_sb = small.tile([CR, BPC], f32)
            gS_sb = small.tile([BPC, C], f32)
            G_sb = small.tile([P, C], f32)

            # weights + constants
            nc.sync.dma_start(w1s[:, 0, :], w1_d[0:P, :])
            nc.sync.dma_start(w1s[:, 1, :], w1_d[P : 2 * P, :])
            nc.sync.dma_start(w2s[:], w2_d[:])
            nc.sync.dma_start(mask[:], mask_d[:])
            nc.sync.dma_start(rt[:], rt_d[:])

            # phase 1: load x chunks; contiguous pairwise tree-adds per chunk
            # (unit-stride DVE runs ~2x faster than a strided tensor_reduce)
            for k in range(NFULL):
                xc = X[:, k * CHUNK : (k + 1) * CHUNK, :]
                eng = nc.sync if k % 2 == 0 else nc.scalar
                eng.dma_start(xc, x_d[:, k * CHUNK : (k + 1) * CHUNK, :])
                tmp = small.tile([P, CHUNK // 2, C], f32, tag="tree")
                h = CHUNK // 2
                nc.vector.tensor_tensor(tmp[:, 0:h, :], xc[:, 0:h, :], xc[:, h : 2 * h, :], op=op.add)
                while h > 2:
                    h //= 2
                    nc.vector.tensor_tensor(tmp[:, 0:h, :], tmp[:, 0:h, :], tmp[:, h : 2 * h, :], op=op.add)
                final = acc[:] if k == 0 else tmp[:, 0, :]
                nc.vector.tensor_tensor(final, tmp[:, 0, :], tmp[:, 1, :], op=op.add)
                if k > 0:
                    nc.vector.tensor_tensor(acc[:], acc[:], tmp[:, 0, :], op=op.add)
            # leftover tokens (loaded last so the phase-1 DVE tail is tiny)
            xr = X[:, NFULL * CHUNK : TPP, :]
            nc.sync.dma_start(xr, x_d[:, NFULL * CHUNK : TPP, :])
            tmp = small.tile([P, CHUNK // 2, C], f32, tag="tree")
            nc.vector.tensor_tensor(tmp[:, 0, :], xr[:, 0, :], xr[:, 1, :], op=op.add)
            nc.vector.tensor_tensor(acc[:], acc[:], tmp[:, 0, :], op=op.add)

            # sT[c, b] = sum_p acc[p, c] * mask[p, b]  (= gap mean, transposed)
            sT_ps = psum.tile([P, 2 * BPC], f32)
            nc.tensor.matmul(sT_ps[:, 0:BPC], acc[:, 0:P], mask[:], start=True, stop=True)
            nc.tensor.matmul(sT_ps[:, BPC : 2 * BPC], acc[:, P : 2 * P], mask[:], start=True, stop=True)
            nc.scalar.copy(sT_sb[:], sT_ps[:])

            # zT[r, b] = sum_c w1[c, r] * sT[c, b]
            zT_ps = psum.tile([CR, BPC], f32)
            nc.tensor.matmul(zT_ps[:], w1s[:, 0, :], sT_sb[:, 0:BPC], start=True, stop=False)
            nc.tensor.matmul(zT_ps[:], w1s[:, 1, :], sT_sb[:, BPC : 2 * BPC], start=False, stop=True)
            # relu6
            nc.vector.tensor_scalar(zT_sb[:], zT_ps[:], 0.0, 6.0, op0=op.max, op1=op.min)

            # y[b, c] = sum_r zT[r, b] * w2[r, c]
            yS_ps = psum.tile([BPC, C], f32)
            nc.tensor.matmul(yS_ps[:], zT_sb[:], w2s[:], start=True, stop=True)
            # hsigmoid: min(max(y + 3, 0), 6) / 6
            nc.vector.tensor_scalar(gS_sb[:], yS_ps[:], 3.0, 0.0, op0=op.add, op1=op.max)
            nc.vector.tensor_scalar(gS_sb[:], gS_sb[:], 6.0, 1.0 / 6.0, op0=op.min, op1=op.mult)

            # replicate gate rows to all partitions: G[p, c] = g[p // SPP, c]
            G_ps = psum.tile([P, C], f32)
            nc.tensor.matmul(G_ps[:], rt[:], gS_sb[:], start=True, stop=True)
            nc.scalar.copy(G_sb[:], G_ps[:])

            # phase 2: gate multiply in place, stream out.
            # leftover (2-token) chunk first: its multiply is tiny, so the
            # first store is issued almost immediately after the gate is ready.
            xr2 = X[:, NFULL * CHUNK : TPP, :]
            gbr = G_sb[:].unsqueeze(1).broadcast_to([P, REM, C])
            nc.vector.tensor_tensor(xr2, xr2, gbr, op=op.mult)
            nc.scalar.dma_start(o_d[:, NFULL * CHUNK : TPP, :], xr2)
            for k in range(NFULL):
                xc = X[:, k * CHUNK : (k + 1) * CHUNK, :]
                gb = G_sb[:].unsqueeze(1).broadcast_to([P, CHUNK, C])
                nc.vector.tensor_tensor(xc, xc, gb, op=op.mult)
                eng = nc.sync if k % 2 == 0 else nc.scalar
                eng.dma_start(o_d[:, k * CHUNK : (k + 1) * CHUNK, :], xc)

    nc.compile()
    return nc


def _in_maps(x, w1, w2):
    x = np.ascontiguousarray(x, dtype=np.float32)
    w1 = np.ascontiguousarray(w1, dtype=np.float32)
    w2 = np.ascontiguousarray(w2, dtype=np.float32)

    # partition p of the token layout belongs to sample p // SPP
    mask = np.zeros((P, BPC), dtype=np.float32)
    rt = np.zeros((BPC, P), dtype=np.float32)
    for s in range(BPC):
        mask[SPP * s : SPP * (s + 1), s] = 1.0 / TOK
        rt[s, SPP * s : SPP * (s + 1)] = 1.0

    in_maps = []
    for m in range(NCORES):
        shard = x[m * BPC : (m + 1) * BPC].reshape(P, TPP, C)
        in_maps.append({"x": shard, "w1": w1, "w2": w2, "mask": mask, "rt": rt})
    return in_maps


def kernel(x, w1, w2):
    from concourse.bass_utils import run_bass_kernel_spmd

    if "nc" not in _CACHE:
        _CACHE["nc"] = _build()
    nc = _CACHE["nc"]

    res = run_bass_kernel_spmd(nc, _in_maps(x, w1, w2), core_ids=list(range(NCORES)))
    out = np.empty((B, H, W, C), dtype=np.float32)
    for m in range(NCORES):
        out[m * BPC : (m + 1) * BPC] = res.results[m]["out"].reshape(BPC, H, W, C)
    return out


# revision 13
# speedup vs baseline: 1.0854x; 1.0854x over previous
"""HSE (hard squeeze-excite) Trainium2 Bass kernel.

Full inputs: x [32,56,56,256] f32, w1 [256,64], w2 [64,256].
out = x * hsigmoid(relu6(gap(x) @ w1) @ w2), gap = mean over H,W.

Sharding: pure data-parallel over batch, 4 samples per core on 8 cores.
Per-core layout: x shard [4*56*56, 256] tokens viewed as [128, 98, 256]
(partition p holds tokens p*98 .. p*98+97; 3136 = 32*98 so partitions
0-31 -> sample 0, 32-63 -> sample 1, etc. -- each partition line holds
tokens of exactly one sample).
"""

import numpy as np

B, H, W, C = 32, 56, 56, 256
CR = 64
NCORES = 8
BPC = B // NCORES            # 4 samples per core
TOK = H * W                  # 3136 tokens per sample
P = 128                      # SBUF partitions
TPP = BPC * TOK // P         # 98 tokens per partition
SPP = P // BPC               # 32 partitions per sample
CHUNK = 16                   # tokens per pipeline chunk (pow2 for tree adds)
NFULL = TPP // CHUNK         # 6 full chunks
REM = TPP - NFULL * CHUNK    # 2 leftover tokens

_CACHE = {}


def _build():
    import concourse.bacc as bacc
    import concourse.tile as tile
    import concourse.mybir as mybir

    f32 = mybir.dt.float32
    op = mybir.AluOpType

    nc = bacc.Bacc("TRN2", target_bir_lowering=False, debug=False)

    x_d = nc.dram_tensor("x", [P, TPP, C], f32, kind="ExternalInput").ap()
    w1_d = nc.dram_tensor("w1", [C, CR], f32, kind="ExternalInput").ap()
    w2_d = nc.dram_tensor("w2", [CR, C], f32, kind="ExternalInput").ap()
    mask_d = nc.dram_tensor("mask", [P, BPC], f32, kind="ExternalInput").ap()
    rt_d = nc.dram_tensor("rt", [BPC, P], f32, kind="ExternalInput").ap()
    o_d = nc.dram_tensor("out", [P, TPP, C], f32, kind="ExternalOutput").ap()

    with tile.TileContext(nc) as tc:
        with tc.tile_pool(name="big", bufs=1) as big, \
             tc.tile_pool(name="small", bufs=1) as small, \
             tc.tile_pool(name="psum", bufs=1, space="PSUM") as psum:

            X = big.tile([P, TPP, C], f32)          # whole shard, ~100KB/part
            acc = small.tile([P, C], f32)           # per-partition token sums
            w1s = small.tile([P, 2, CR], f32)       # w1 rows split in halves
            w2s = small.tile([CR, C], f32)
            mask = small.tile([P, BPC], f32)        # partition->sample (1/TOK)
            rt = small.tile([BPC, P], f32)          # sample->partition replicate
            sT_sb = small.tile([P, 2 * BPC], f32)
            zT_sb = small.tile([CR, BPC], f32)
            gS_sb = small.tile([BPC, C], f32)
            G_sb = small.tile([P, C], f32)

            # weights + constants on the scalar HWDGE ring: the sync ring is
            # reserved for the x chunk loads so they drain strictly in order
            nc.scalar.dma_start(w1s[:, 0, :], w1_d[0:P, :])
            nc.scalar.dma_start(w1s[:, 1, :], w1_d[P : 2 * P, :])
            nc.scalar.dma_start(w2s[:], w2_d[:])
            nc.scalar.dma_start(mask[:], mask_d[:])
            nc.scalar.dma_start(rt[:], rt_d[:])

            # phase 1: load x chunks; contiguous pairwise tree-adds per chunk
            # (unit-stride DVE runs ~2x faster than a strided tensor_reduce)
            for k in range(NFULL):
                xc = X[:, k * CHUNK : (k + 1) * CHUNK, :]
                nc.sync.dma_start(xc, x_d[:, k * CHUNK : (k + 1) * CHUNK, :])
                tmp = small.tile([P, CHUNK // 2, C], f32, tag="tree")
                h = CHUNK // 2
                nc.vector.tensor_tensor(tmp[:, 0:h, :], xc[:, 0:h, :], xc[:, h : 2 * h, :], op=op.add)
                while h > 2:
                    h //= 2
                    nc.vector.tensor_tensor(tmp[:, 0:h, :], tmp[:, 0:h, :], tmp[:, h : 2 * h, :], op=op.add)
                final = acc[:] if k == 0 else tmp[:, 0, :]
                nc.vector.tensor_tensor(final, tmp[:, 0, :], tmp[:, 1, :], op=op.add)
                if k > 0:
                    nc.vector.tensor_tensor(acc[:], acc[:], tmp[:, 0, :], op=op.add)
            # leftover tokens (loaded last so the phase-1 DVE tail is tiny)
            xr = X[:, NFULL * CHUNK : TPP, :]
            nc.sync.dma_start(xr, x_d[:, NFULL * CHUNK : TPP, :])
            tmp = small.tile([P, CHUNK // 2, C], f32, tag="tree")
            nc.vector.tensor_tensor(tmp[:, 0, :], xr[:, 0, :], xr[:, 1, :], op=op.add)
            nc.vector.tensor_tensor(acc[:], acc[:], tmp[:, 0, :], op=op.add)

            # sT[c, b] = sum_p acc[p, c] * mask[p, b]  (= gap mean, transposed)
            sT_ps = psum.tile([P, 2 * BPC], f32)
            nc.tensor.matmul(sT_ps[:, 0:BPC], acc[:, 0:P], mask[:], start=True, stop=True)
            nc.tensor.matmul(sT_ps[:, BPC : 2 * BPC], acc[:, P : 2 * P], mask[:], start=True, stop=True)
            nc.scalar.copy(sT_sb[:], sT_ps[:])

            # zT[r, b] = sum_c w1[c, r] * sT[c, b]
            zT_ps = psum.tile([CR, BPC], f32)
            nc.tensor.matmul(zT_ps[:], w1s[:, 0, :], sT_sb[:, 0:BPC], start=True, stop=False)
            nc.tensor.matmul(zT_ps[:], w1s[:, 1, :], sT_sb[:, BPC : 2 * BPC], start=False, stop=True)
            # relu6
            nc.vector.tensor_scalar(zT_sb[:], zT_ps[:], 0.0, 6.0, op0=op.max, op1=op.min)

            # y[b, c] = sum_r zT[r, b] * w2[r, c]
            yS_ps = psum.tile([BPC, C], f32)
            nc.tensor.matmul(yS_ps[:], zT_sb[:], w2s[:], start=True, stop=True)
            # hsigmoid: min(max(y + 3, 0), 6) / 6
            nc.vector.tensor_scalar(gS_sb[:], yS_ps[:], 3.0, 0.0, op0=op.add, op1=op.max)
            nc.vector.tensor_scalar(gS_sb[:], gS_sb[:], 6.0, 1.0 / 6.0, op0=op.min, op1=op.mult)

            # replicate gate rows to all partitions: G[p, c] = g[p // SPP, c]
            G_ps = psum.tile([P, C], f32)
            nc.tensor.matmul(G_ps[:], rt[:], gS_sb[:], start=True, stop=True)
            nc.scalar.copy(G_sb[:], G_ps[:])

            # phase 2: gate multiply in place, stream out.
            # leftover (2-token) chunk first: its multiply is tiny, so the
            # first store is issued almost immediately after the gate is ready.
            xr2 = X[:, NFULL * CHUNK : TPP, :]
            gbr = G_sb[:].unsqueeze(1).broadcast_to([P, REM, C])
            nc.vector.tensor_tensor(xr2, xr2, gbr, op=op.mult)
            nc.scalar.dma_start(o_d[:, NFULL * CHUNK : TPP, :], xr2)
            for k in range(NFULL):
                xc = X[:, k * CHUNK : (k + 1) * CHUNK, :]
                gb = G_sb[:].unsqueeze(1).broadcast_to([P, CHUNK, C])
                nc.vector.tensor_tensor(xc, xc, gb, op=op.mult)
                eng = nc.sync if k % 2 == 0 else nc.scalar
                eng.dma_start(o_d[:, k * CHUNK : (k + 1) * CHUNK, :], xc)

    nc.compile()
    return nc


def _in_maps(x, w1, w2):
    x = np.ascontiguousarray(x, dtype=np.float32)
    w1 = np.ascontiguousarray(w1, dtype=np.float32)
    w2 = np.ascontiguousarray(w2, dtype=np.float32)

    # partition p of the token layout belongs to sample p // SPP
    mask = np.zeros((P, BPC), dtype=np.float32)
    rt = np.zeros((BPC, P), dtype=np.float32)
    for s in range(BPC):
        mask[SPP * s : SPP * (s + 1), s] = 1.0 / TOK
        rt[s, SPP * s : SPP * (s + 1)] = 1.0

    in_maps = []
    for m in range(NCORES):
        shard = x[m * BPC : (m + 1) * BPC].reshape(P, TPP, C)
        in_maps.append({"x": shard, "w1": w1, "w2": w2, "mask": mask, "rt": rt})
    return in_maps


def kernel(x, w1, w2):
    from concourse.bass_utils import run_bass_kernel_spmd

    if "nc" not in _CACHE:
        _CACHE["nc"] = _build()
    nc = _CACHE["nc"]

    res = run_bass_kernel_spmd(nc, _in_maps(x, w1, w2), core_ids=list(range(NCORES)))
    out = np.empty((B, H, W, C), dtype=np.float32)
    for m in range(NCORES):
        out[m * BPC : (m + 1) * BPC] = res.results[m]["out"].reshape(BPC, H, W, C)
    return out
